# revision 14
# baseline (speedup 1.0000x reference)
"""MoE low-rank adapters (top-1 routing) Trainium2 kernel.

Math (reference):
  xf = x.reshape(N, D)                 N=8192, D=2048, E=8, R=64
  logits = xf @ Wg.T                   [N, E]
  prob = softmax(logits); gate = argmax(prob); prob_sel = max(prob)
  h = xf @ A[e].T for all e            [N, E*R]
  y = (h * onehot(gate)) @ Bwt         [N, D]
  y *= SCALING * prob_sel

Distribution: data-parallel over tokens, 8 cores x 1024 tokens.

Per-core device pipeline (4 blocks of 256 tokens):
  - gating logits^T via col-tiled exact-fp32 matmuls (stationary Wg^T chunks
    [128,8] at 4 PE column groups; moving x^T fp32)
  - h^T = A_t.T @ x_hi in bf16 (A_t host-transposed bf16; x_hi on-chip cast)
  - softmax/argmax epilogue in natural layout after tiny PE transposes;
    mask*scale coefficient maskval[tok,e] = (logit==max) * SCALING/sumexp
  - maskval transposed (PE) -> staged to DRAM -> partition-broadcast DMA
    to expand over the 64 ranks of each expert -> h_masked^T (f32r)
  - y = h_masked^T.T @ Bwt in f32r (Bwt host-transposed, f32r-typed)
"""

import sys
import os

for _p in ("/opt/trn_rl_repo",):
    if _p not in sys.path:
        sys.path.insert(0, _p)

import numpy as np
import ml_dtypes

import concourse.bass as bass
import concourse.bacc as bacc
import concourse.mybir as mybir
import concourse.tile as tile
from concourse import bass_utils
from concourse.masks import make_identity

f32 = mybir.dt.float32
f32r = mybir.dt.float32r
bf16 = mybir.dt.bfloat16

B, S, D, R, E = 4, 2048, 2048, 64, 8
N = B * S                    # 8192 tokens
NCORES = 8
NTOK = N // NCORES           # 1024 tokens per core
SCALING = 64.0 / 16.0
ER = E * R                   # 512
KD = D // 128                # 16 d-chunks
BLK = 256                    # tokens per block
NBLK = NTOK // BLK           # 4 blocks
TCH = BLK // 128             # tok-chunks per block (2)
NOCH = 4                     # output chunks of 512
ERCH = ER // 128             # er chunks (4)

_CACHE = {}


def _build():
    if "nc" in _CACHE:
        return _CACHE["nc"]
    nc = bacc.Bacc("TRN2", target_bir_lowering=False, debug=False,
                   num_devices=NCORES)
    xt = nc.dram_tensor("xt", [D, NTOK], f32, kind="ExternalInput")
    abf = nc.dram_tensor("abf", [D, ER], bf16, kind="ExternalInput")
    bwr = nc.dram_tensor("bwr", [ER, D], f32r, kind="ExternalInput")
    wg = nc.dram_tensor("wg", [D, E], f32, kind="ExternalInput")
    sel = nc.dram_tensor("sel", [128, E], f32, kind="ExternalInput")
    yo = nc.dram_tensor("yo", [NTOK, D], f32, kind="ExternalOutput")
    mstage = nc.dram_tensor("mstage", [NBLK, E, BLK], f32, kind="Internal")

    with tile.TileContext(nc) as tc:
        import contextlib
        ctx = contextlib.ExitStack()
        with ctx:
            singles = ctx.enter_context(tc.tile_pool(name="singles", bufs=1))
            xpool = ctx.enter_context(tc.tile_pool(name="xpool", bufs=3))
            hpool = ctx.enter_context(tc.tile_pool(name="hpool", bufs=2))
            mpool = ctx.enter_context(tc.tile_pool(name="mpool", bufs=2))
            spool = ctx.enter_context(tc.tile_pool(name="spool", bufs=3))
            ypool = ctx.enter_context(tc.tile_pool(name="ypool", bufs=3))
            ps_h = ctx.enter_context(tc.tile_pool(name="ps_h", bufs=1, space="PSUM"))
            ps_lg = ctx.enter_context(tc.tile_pool(name="ps_lg", bufs=1, space="PSUM"))
            ps_tr = ctx.enter_context(tc.tile_pool(name="ps_tr", bufs=1, space="PSUM"))
            ps_y = ctx.enter_context(tc.tile_pool(name="ps_y", bufs=2, space="PSUM"))

            # ---- weights (loaded once) ----
            # gating weights + selector first (block 0's x load follows on the
            # same sync queue); big adapter weights go on the scalar HWDGE
            # queue so they don't serialize in front of x.
            wg_sb = singles.tile([128, KD, E], f32)
            nc.sync.dma_start(out=wg_sb,
                              in_=wg.ap().rearrange("(k p) e -> p k e", p=128))
            sel_sb = singles.tile([128, E], f32)
            nc.sync.dma_start(out=sel_sb, in_=sel.ap())
            abf_sb = singles.tile([128, KD, ER], bf16)
            nc.scalar.dma_start(out=abf_sb,
                                in_=abf.ap().rearrange("(k p) e -> p k e", p=128))
            bwr_sb = singles.tile([128, ERCH, D], f32r)
            nc.scalar.dma_start(out=bwr_sb,
                                in_=bwr.ap().rearrange("(i p) o -> p i o", p=128))
            ident = singles.tile([128, 128], f32)
            make_identity(nc, ident)

            # ---- x^T resident in SBUF, one DMA per d-chunk (4KB runs) ----
            xbig = singles.tile([128, KD, NTOK], f32)
            for k in range(KD):
                nc.sync.dma_start(
                    out=xbig[:, k, :],
                    in_=xt.ap()[128 * k:128 * k + 128, :])

            for blk in range(NBLK):
                n0 = blk * BLK
                xt32 = xbig[:, :, n0:n0 + BLK]
                # ---- cast to bf16 for the h matmul ----
                xhi = xpool.tile([128, KD, BLK], bf16, tag="xhi")
                nc.scalar.copy(xhi, xt32)

                # ---- gating: col-tiled exact fp32, logits^T accumulation ----
                # group-major order: 4 chunks per PE column-group before
                # switching tile_position (switches cost an array drain)
                lg_ps = ps_lg.tile([128, BLK], f32, tag="lg")
                for g in range(4):
                    for r in range(KD // 4):
                        k = 4 * r + g
                        nc.tensor.matmul(
                            lg_ps[32 * g:32 * g + E, :],
                            wg_sb[:, k, :],
                            xt32[:, k, :],
                            start=(r == 0), stop=(r == KD // 4 - 1),
                            tile_position=(0, 32 * g),
                        )

                # ---- h^T = A_t.T @ x_hi (bf16), 4 er-chunks ----
                h_ps = []
                for i in range(ERCH):
                    hp = ps_h.tile([128, BLK], f32, tag=f"h{i}")
                    h_ps.append(hp)
                    for k in range(KD):
                        nc.tensor.matmul(
                            hp, abf_sb[:, k, 128 * i:128 * i + 128],
                            xhi[:, k, :],
                            start=(k == 0), stop=(k == KD - 1))

                # ---- gating epilogue ----
                # copy col-group partials to SBUF; the 4-group reduction and
                # transpose to natural layout fuse into one matmul with SEL
                # (SEL[p, e] = 1 iff p % 32 == e):
                #   lg_nat[tok, e] = sum_p lg_sb[p, tok] * SEL[p, e]
                lg_sb = spool.tile([128, BLK], f32, tag="lg_sb")
                nc.scalar.copy(lg_sb, lg_ps)

                # per tok-chunk: natural-layout logits, softmax bits, maskval
                mvT = mpool.tile([8, BLK], f32, tag="mvT")
                for t in range(TCH):
                    trp = ps_tr.tile([128, 8], f32, tag="tr")
                    nc.tensor.matmul(trp, lg_sb[:, 128 * t:128 * t + 128],
                                     sel_sb, start=True, stop=True)
                    lg = spool.tile([128, 8], f32, tag="lg_nat")
                    nc.scalar.copy(lg, trp)
                    mx = spool.tile([128, 1], f32, tag="mx")
                    nc.vector.reduce_max(out=mx, in_=lg, axis=mybir.AxisListType.X)
                    negmx = spool.tile([128, 1], f32, tag="negmx")
                    nc.vector.tensor_scalar_mul(negmx, mx, -1.0)
                    es = spool.tile([128, 8], f32, tag="es")
                    se = spool.tile([128, 1], f32, tag="se")
                    nc.scalar.activation(out=es, in_=lg,
                                         func=mybir.ActivationFunctionType.Exp,
                                         bias=negmx, scale=1.0, accum_out=se)
                    rcp = spool.tile([128, 1], f32, tag="rcp")
                    nc.vector.reciprocal(rcp, se)
                    rsc = spool.tile([128, 1], f32, tag="rsc")
                    nc.vector.tensor_scalar_mul(rsc, rcp, SCALING)
                    mval = spool.tile([128, 8], f32, tag="mval")
                    nc.vector.tensor_scalar(
                        out=mval, in0=lg, scalar1=mx, scalar2=rsc,
                        op0=mybir.AluOpType.is_equal, op1=mybir.AluOpType.mult)
                    # transpose maskval -> [8, 128] and collect into mvT
                    mtr = ps_tr.tile([8, 128], f32, tag="tr")
                    nc.tensor.transpose(mtr, mval, ident)
                    nc.scalar.copy(mvT[:, 128 * t:128 * t + 128], mtr)

                # stage maskval^T to DRAM, broadcast back over expert ranks
                # (gpsimd SWDGE queue: decouples from bulk x/y traffic)
                nc.gpsimd.dma_start(out=mstage.ap()[blk], in_=mvT)
                mexp = []
                for i in range(ERCH):
                    me = mpool.tile([128, BLK], f32, tag=f"me{i}")
                    mexp.append(me)
                    src = bass.AP(
                        tensor=mstage,
                        offset=(blk * E + 2 * i) * BLK,
                        ap=[[BLK, 2], [0, 64], [1, BLK]],
                    )
                    nc.gpsimd.dma_start(out=me, in_=src)

                # ---- apply mask*scale: h_masked^T (f32r) ----
                hmT = []
                for i in range(ERCH):
                    hm = hpool.tile([128, BLK], f32r, tag=f"hm{i}")
                    hmT.append(hm)
                    nc.vector.tensor_mul(hm, h_ps[i], mexp[i])

                # ---- y = hmT.T @ Bwt (f32r) ----
                for t in range(TCH):
                    ysb = ypool.tile([128, D], f32, tag="ysb")
                    for j in range(NOCH):
                        yp = ps_y.tile([128, 512], f32, tag="y")
                        for i in range(ERCH):
                            nc.tensor.matmul(
                                yp, hmT[i][:, 128 * t:128 * t + 128],
                                bwr_sb[:, i, 512 * j:512 * j + 512],
                                start=(i == 0), stop=(i == ERCH - 1))
                        if j % 2 == 0:
                            nc.scalar.copy(ysb[:, 512 * j:512 * j + 512], yp)
                        else:
                            nc.vector.tensor_copy(ysb[:, 512 * j:512 * j + 512], yp)
                    nc.sync.dma_start(
                        out=yo.ap()[n0 + 128 * t:n0 + 128 * t + 128, :],
                        in_=ysb)

    nc.compile()
    _CACHE["nc"] = nc
    return nc


def _prep_inputs(x, A, Bw, Wg):
    xf = np.ascontiguousarray(np.asarray(x, dtype=np.float32).reshape(N, D))
    xT = np.ascontiguousarray(xf.T)                              # [D, N]
    A_t = np.ascontiguousarray(
        np.asarray(A, dtype=np.float32).reshape(ER, D).T).astype(ml_dtypes.bfloat16)
    Bwt = np.ascontiguousarray(
        np.asarray(Bw, dtype=np.float32).transpose(0, 2, 1).reshape(ER, D))
    WgT = np.ascontiguousarray(np.asarray(Wg, dtype=np.float32).T)  # [D, E]
    SEL = np.zeros((128, E), dtype=np.float32)
    for p in range(128):
        if p % 32 < E:
            SEL[p, p % 32] = 1.0
    in_maps = []
    for c in range(NCORES):
        in_maps.append({
            "xt": np.ascontiguousarray(xT[:, c * NTOK:(c + 1) * NTOK]),
            "abf": A_t,
            "bwr": Bwt,
            "wg": WgT,
            "sel": SEL,
        })
    return in_maps


def _run(x, A, Bw, Wg, trace=False):
    nc = _build()
    in_maps = _prep_inputs(x, A, Bw, Wg)
    res = bass_utils.run_bass_kernel_spmd(
        nc, in_maps, core_ids=list(range(NCORES)), trace=trace)
    y = np.concatenate([res.results[c]["yo"] for c in range(NCORES)], axis=0)
    return y.reshape(B, S, D), res


def kernel(x, A, Bw, Wg):
    y, _ = _run(x, A, Bw, Wg, trace=False)
    return y


# revision 17
# speedup vs baseline: 1.0261x; 1.0261x over previous
"""MoE low-rank adapters (top-1 routing) Trainium2 kernel.

Math (reference):
  xf = x.reshape(N, D)                 N=8192, D=2048, E=8, R=64
  logits = xf @ Wg.T                   [N, E]
  prob = softmax(logits); gate = argmax(prob); prob_sel = max(prob)
  h = xf @ A[e].T for all e            [N, E*R]
  y = (h * onehot(gate)) @ Bwt         [N, D]
  y *= SCALING * prob_sel

Distribution: data-parallel over tokens, 8 cores x 1024 tokens.

Per-core device pipeline (4 blocks of 256 tokens):
  - gating logits^T via col-tiled exact-fp32 matmuls (stationary Wg^T chunks
    [128,8] at 4 PE column groups; moving x^T fp32)
  - h^T = A_t.T @ x_hi in bf16 (A_t host-transposed bf16; x_hi on-chip cast)
  - softmax/argmax epilogue in natural layout after tiny PE transposes;
    mask*scale coefficient maskval[tok,e] = (logit==max) * SCALING/sumexp
  - maskval transposed (PE) -> staged to DRAM -> partition-broadcast DMA
    to expand over the 64 ranks of each expert -> h_masked^T (f32r)
  - y = h_masked^T.T @ Bwt in f32r (Bwt host-transposed, f32r-typed)
"""

import sys
import os

for _p in ("/opt/trn_rl_repo",):
    if _p not in sys.path:
        sys.path.insert(0, _p)

import numpy as np
import ml_dtypes

import concourse.bass as bass
import concourse.bacc as bacc
import concourse.mybir as mybir
import concourse.tile as tile
from concourse import bass_utils
from concourse.masks import make_identity

f32 = mybir.dt.float32
f32r = mybir.dt.float32r
bf16 = mybir.dt.bfloat16

B, S, D, R, E = 4, 2048, 2048, 64, 8
N = B * S                    # 8192 tokens
NCORES = 8
NTOK = N // NCORES           # 1024 tokens per core
SCALING = 64.0 / 16.0
ER = E * R                   # 512
KD = D // 128                # 16 d-chunks
BLK = 256                    # tokens per block
NBLK = NTOK // BLK           # 4 blocks
TCH = BLK // 128             # tok-chunks per block (2)
NOCH = 4                     # output chunks of 512
ERCH = ER // 128             # er chunks (4)

_CACHE = {}


def _build():
    if "nc" in _CACHE:
        return _CACHE["nc"]
    nc = bacc.Bacc("TRN2", target_bir_lowering=False, debug=False,
                   num_devices=NCORES)
    xt = nc.dram_tensor("xt", [D, NTOK], f32, kind="ExternalInput")
    abf = nc.dram_tensor("abf", [D, ER], bf16, kind="ExternalInput")
    bwr = nc.dram_tensor("bwr", [ER, D], f32r, kind="ExternalInput")
    wg = nc.dram_tensor("wg", [D, E], f32, kind="ExternalInput")
    sel = nc.dram_tensor("sel", [128, E], f32, kind="ExternalInput")
    yo = nc.dram_tensor("yo", [NTOK, D], f32, kind="ExternalOutput")
    mstage = nc.dram_tensor("mstage", [NBLK, E, BLK], f32, kind="Internal")

    with tile.TileContext(nc) as tc:
        import contextlib
        ctx = contextlib.ExitStack()
        with ctx:
            singles = ctx.enter_context(tc.tile_pool(name="singles", bufs=1))
            xpool = ctx.enter_context(tc.tile_pool(name="xpool", bufs=3))
            hpool = ctx.enter_context(tc.tile_pool(name="hpool", bufs=2))
            mpool = ctx.enter_context(tc.tile_pool(name="mpool", bufs=2))
            spool = ctx.enter_context(tc.tile_pool(name="spool", bufs=3))
            ypool = ctx.enter_context(tc.tile_pool(name="ypool", bufs=3))
            ps_h = ctx.enter_context(tc.tile_pool(name="ps_h", bufs=1, space="PSUM"))
            ps_lg = ctx.enter_context(tc.tile_pool(name="ps_lg", bufs=1, space="PSUM"))
            ps_tr = ctx.enter_context(tc.tile_pool(name="ps_tr", bufs=1, space="PSUM"))
            ps_y = ctx.enter_context(tc.tile_pool(name="ps_y", bufs=2, space="PSUM"))

            # ---- weights (loaded once) ----
            # gating weights + selector first (block 0's x load follows on the
            # same sync queue); big adapter weights go on the scalar HWDGE
            # queue so they don't serialize in front of x.
            wg_sb = singles.tile([128, KD, E], f32)
            nc.sync.dma_start(out=wg_sb,
                              in_=wg.ap().rearrange("(k p) e -> p k e", p=128))
            sel_sb = singles.tile([128, E], f32)
            nc.sync.dma_start(out=sel_sb, in_=sel.ap())
            ident = singles.tile([128, 128], f32)
            make_identity(nc, ident)

            # ---- x^T resident in SBUF, one DMA per d-chunk (4KB runs) ----
            # chunk order matches gating group-major consumption; chunks
            # alternate between the two HWDGE queues. Adapter weights queue
            # behind x on the scalar queue (h starts after gating, y later).
            KORDER = [4 * r + g for g in range(4) for r in range(4)]
            xbig = singles.tile([128, KD, NTOK], f32)
            abf_sb = singles.tile([128, KD, ER], bf16)
            bwr_sb = singles.tile([128, ERCH, D], f32r)
            for idx, k in enumerate(KORDER):
                eng = nc.sync if idx % 2 == 0 else nc.scalar
                eng.dma_start(
                    out=xbig[:, k, :],
                    in_=xt.ap()[128 * k:128 * k + 128, :])
            nc.scalar.dma_start(out=abf_sb,
                                in_=abf.ap().rearrange("(k p) e -> p k e", p=128))
            nc.scalar.dma_start(out=bwr_sb,
                                in_=bwr.ap().rearrange("(i p) o -> p i o", p=128))

            for blk in range(NBLK):
                n0 = blk * BLK
                xt32 = xbig[:, :, n0:n0 + BLK]
                # ---- cast to bf16 for the h matmul (per-chunk, in x arrival
                # order so block 0's h work can start early) ----
                xhi = xpool.tile([128, KD, BLK], bf16, tag="xhi")
                for k in KORDER:
                    nc.scalar.copy(xhi[:, k, :], xt32[:, k, :])

                # ---- gating: col-tiled exact fp32, logits^T accumulation ----
                # group-major order: 4 chunks per PE column-group before
                # switching tile_position (switches cost an array drain)
                lg_ps = ps_lg.tile([128, BLK], f32, tag="lg")
                for g in range(4):
                    for r in range(KD // 4):
                        k = 4 * r + g
                        nc.tensor.matmul(
                            lg_ps[32 * g:32 * g + E, :],
                            wg_sb[:, k, :],
                            xt32[:, k, :],
                            start=(r == 0), stop=(r == KD // 4 - 1),
                            tile_position=(0, 32 * g),
                        )

                # ---- h^T = A_t.T @ x_hi (bf16), 4 er-chunks ----
                h_ps = []
                for i in range(ERCH):
                    hp = ps_h.tile([128, BLK], f32, tag=f"h{i}")
                    h_ps.append(hp)
                    for kk, k in enumerate(KORDER):
                        nc.tensor.matmul(
                            hp, abf_sb[:, k, 128 * i:128 * i + 128],
                            xhi[:, k, :],
                            start=(kk == 0), stop=(kk == KD - 1))

                # ---- gating epilogue ----
                # copy col-group partials to SBUF; the 4-group reduction and
                # transpose to natural layout fuse into one matmul with SEL
                # (SEL[p, e] = 1 iff p % 32 == e):
                #   lg_nat[tok, e] = sum_p lg_sb[p, tok] * SEL[p, e]
                lg_sb = spool.tile([128, BLK], f32, tag="lg_sb")
                nc.scalar.copy(lg_sb, lg_ps)

                # per tok-chunk: natural-layout logits, softmax bits, maskval
                mvT = mpool.tile([8, BLK], f32, tag="mvT")
                for t in range(TCH):
                    trp = ps_tr.tile([128, 8], f32, tag="tr")
                    nc.tensor.matmul(trp, lg_sb[:, 128 * t:128 * t + 128],
                                     sel_sb, start=True, stop=True)
                    lg = spool.tile([128, 8], f32, tag="lg_nat")
                    nc.scalar.copy(lg, trp)
                    mx = spool.tile([128, 1], f32, tag="mx")
                    nc.vector.reduce_max(out=mx, in_=lg, axis=mybir.AxisListType.X)
                    negmx = spool.tile([128, 1], f32, tag="negmx")
                    nc.vector.tensor_scalar_mul(negmx, mx, -1.0)
                    es = spool.tile([128, 8], f32, tag="es")
                    se = spool.tile([128, 1], f32, tag="se")
                    nc.scalar.activation(out=es, in_=lg,
                                         func=mybir.ActivationFunctionType.Exp,
                                         bias=negmx, scale=1.0, accum_out=se)
                    rcp = spool.tile([128, 1], f32, tag="rcp")
                    nc.vector.reciprocal(rcp, se)
                    rsc = spool.tile([128, 1], f32, tag="rsc")
                    nc.vector.tensor_scalar_mul(rsc, rcp, SCALING)
                    mval = spool.tile([128, 8], f32, tag="mval")
                    nc.vector.tensor_scalar(
                        out=mval, in0=lg, scalar1=mx, scalar2=rsc,
                        op0=mybir.AluOpType.is_equal, op1=mybir.AluOpType.mult)
                    # transpose maskval -> [8, 128] and collect into mvT
                    mtr = ps_tr.tile([8, 128], f32, tag="tr")
                    nc.tensor.transpose(mtr, mval, ident)
                    nc.scalar.copy(mvT[:, 128 * t:128 * t + 128], mtr)

                # stage maskval^T to DRAM, broadcast back over expert ranks
                # (gpsimd SWDGE queue: decouples from bulk x/y traffic)
                nc.gpsimd.dma_start(out=mstage.ap()[blk], in_=mvT)
                mexp = []
                for i in range(ERCH):
                    me = mpool.tile([128, BLK], f32, tag=f"me{i}")
                    mexp.append(me)
                    src = bass.AP(
                        tensor=mstage,
                        offset=(blk * E + 2 * i) * BLK,
                        ap=[[BLK, 2], [0, 64], [1, BLK]],
                    )
                    nc.gpsimd.dma_start(out=me, in_=src)

                # ---- apply mask*scale: h_masked^T (f32r) ----
                hmT = []
                for i in range(ERCH):
                    hm = hpool.tile([128, BLK], f32r, tag=f"hm{i}")
                    hmT.append(hm)
                    nc.vector.tensor_mul(hm, h_ps[i], mexp[i])

                # ---- y = hmT.T @ Bwt (f32r) ----
                for t in range(TCH):
                    ysb = ypool.tile([128, D], f32, tag="ysb")
                    for j in range(NOCH):
                        yp = ps_y.tile([128, 512], f32, tag="y")
                        for i in range(ERCH):
                            nc.tensor.matmul(
                                yp, hmT[i][:, 128 * t:128 * t + 128],
                                bwr_sb[:, i, 512 * j:512 * j + 512],
                                start=(i == 0), stop=(i == ERCH - 1))
                        if j % 2 == 0:
                            nc.scalar.copy(ysb[:, 512 * j:512 * j + 512], yp)
                        else:
                            nc.vector.tensor_copy(ysb[:, 512 * j:512 * j + 512], yp)
                    nc.sync.dma_start(
                        out=yo.ap()[n0 + 128 * t:n0 + 128 * t + 128, :],
                        in_=ysb)

    nc.compile()
    _CACHE["nc"] = nc
    return nc


def _prep_inputs(x, A, Bw, Wg):
    xf = np.ascontiguousarray(np.asarray(x, dtype=np.float32).reshape(N, D))
    xT = np.ascontiguousarray(xf.T)                              # [D, N]
    A_t = np.ascontiguousarray(
        np.asarray(A, dtype=np.float32).reshape(ER, D).T).astype(ml_dtypes.bfloat16)
    Bwt = np.ascontiguousarray(
        np.asarray(Bw, dtype=np.float32).transpose(0, 2, 1).reshape(ER, D))
    WgT = np.ascontiguousarray(np.asarray(Wg, dtype=np.float32).T)  # [D, E]
    SEL = np.zeros((128, E), dtype=np.float32)
    for p in range(128):
        if p % 32 < E:
            SEL[p, p % 32] = 1.0
    in_maps = []
    for c in range(NCORES):
        in_maps.append({
            "xt": np.ascontiguousarray(xT[:, c * NTOK:(c + 1) * NTOK]),
            "abf": A_t,
            "bwr": Bwt,
            "wg": WgT,
            "sel": SEL,
        })
    return in_maps


def _run(x, A, Bw, Wg, trace=False):
    nc = _build()
    in_maps = _prep_inputs(x, A, Bw, Wg)
    res = bass_utils.run_bass_kernel_spmd(
        nc, in_maps, core_ids=list(range(NCORES)), trace=trace)
    y = np.concatenate([res.results[c]["yo"] for c in range(NCORES)], axis=0)
    return y.reshape(B, S, D), res


def kernel(x, A, Bw, Wg):
    y, _ = _run(x, A, Bw, Wg, trace=False)
    return y
